# revision 1
# baseline (speedup 1.0000x reference)
"""GraphSAGE x2 + JumpingKnowledge(cat) + Linear block on TRN2, 8-core SPMD.

Host preprocessing shards destination nodes across cores, groups/pads edges,
and builds int16 gather indices; the device kernel does all O(E*F) work:
per-edge feature gather (SWDGE dma_gather), segment-sum via one-hot matmuls
into PSUM, SAGE projections, and the final JK linear.
"""

import math
import numpy as np
import ml_dtypes

import concourse.bass as bass
import concourse.bacc as bacc
import concourse.tile as tile
from concourse import mybir
from concourse.masks import make_identity

P = 128          # partitions / tile height
F = 128          # feature dim (all layers)
K = 4            # number of src chunks (int16 index limit)

f32 = mybir.dt.float32
bf16 = mybir.dt.bfloat16
i16 = mybir.dt.int16


# ----------------------------------------------------------------------------
# Host preprocessing
# ----------------------------------------------------------------------------

class Meta:
    pass


def preprocess(edge_index: np.ndarray, N: int, ncores: int, group_tiles: int):
    """Build per-core padded edge streams with an SPMD-uniform structure.

    Returns Meta with:
      TPC, NPAD, CH, S [TPC,K], groups (list of tile ranges), per-(g,k) slot
      counts, idx16 [ncores][128, IDXCOLS], dstloc [ncores][128, SLOT_TOT],
      invcnt [ncores][128, TPC], col/offset tables.
    """
    m = Meta()
    TPC = math.ceil(N / (ncores * P))
    NPAD = ncores * TPC * P
    CH = math.ceil(NPAD / (K * P)) * P
    assert CH <= 32768
    m.N, m.ncores, m.TPC, m.NPAD, m.CH = N, ncores, TPC, NPAD, CH

    src = edge_index[0].astype(np.int64)
    dst = edge_index[1].astype(np.int64)
    E = src.shape[0]

    core = dst // (TPC * P)
    t_loc = (dst % (TPC * P)) // P
    d_loc = dst % P
    kk = src // CH
    src_rel = (src - kk * CH).astype(np.int16)

    key = ((core * TPC + t_loc) * K + kk).astype(np.int64)
    order = np.argsort(key, kind="stable")
    src_rel_s = src_rel[order]
    d_loc_s = d_loc[order].astype(np.float32)
    counts = np.bincount(key, minlength=ncores * TPC * K).reshape(ncores, TPC, K)
    starts = np.zeros(ncores * TPC * K + 1, dtype=np.int64)
    np.cumsum(counts.reshape(-1), out=starts[1:])

    S = np.ceil(counts / P).astype(np.int64).max(axis=0)      # [TPC, K]
    m.S = S
    slots_t = S.sum(axis=1)                                    # [TPC]
    m.slots_t = slots_t
    m.SLOT_TOT = int(S.sum())

    # groups of tiles
    groups = [list(range(g, min(g + group_tiles, TPC)))
              for g in range(0, TPC, group_tiles)]
    m.groups = groups
    m.n_gk = [[int(S[g, k].sum()) for k in range(K)] for g in
              [np.array(gr) for gr in groups]]                 # slots per (g,k)

    # column offsets
    # dstloc: tile t occupies cols [dcol[t], dcol[t]+slots_t[t])
    dcol = np.zeros(TPC + 1, dtype=np.int64)
    np.cumsum(slots_t, out=dcol[1:])
    m.dcol = dcol
    # within tile t, chunk k starts at slot koff[t,k]
    koff = np.zeros((TPC, K + 1), dtype=np.int64)
    np.cumsum(S, axis=1, out=koff[:, 1:])
    m.koff = koff
    # gather stream: per (g,k), tile t at slot goff[t,k] (within that gather)
    goff = np.zeros((TPC, K), dtype=np.int64)
    for gi, gr in enumerate(groups):
        for k in range(K):
            off = 0
            for t in gr:
                goff[t, k] = off
                off += S[t, k]
    m.goff = goff
    # idx16 col offsets per (g,k): [128, n_gk*8] each
    idxcol = np.zeros((len(groups), K + 1), dtype=np.int64)
    flat = 0
    idxcols_gk = np.zeros((len(groups), K), dtype=np.int64)
    for gi in range(len(groups)):
        for k in range(K):
            idxcols_gk[gi, k] = flat
            flat += m.n_gk[gi][k] * (P // 16)
    m.idxcols_gk = idxcols_gk
    m.IDXCOLS = int(flat)

    # per-core data
    idx16_all, dstloc_all, invcnt_all = [], [], []
    cnt_full = np.bincount(dst, minlength=NPAD).astype(np.float32)
    inv_full = 1.0 / np.maximum(cnt_full, 1.0)
    for c in range(ncores):
        # gather idx stream, per (g,k)
        idx16 = np.zeros((P, m.IDXCOLS), dtype=np.int16)
        dstloc = np.full((P, m.SLOT_TOT), -1.0, dtype=np.float32)
        for gi, gr in enumerate(groups):
            for k in range(K):
                n = m.n_gk[gi][k]
                if n == 0:
                    continue
                stream = np.zeros(n * P, dtype=np.int16)
                for t in gr:
                    s0 = starts[(c * TPC + t) * K + k]
                    s1 = starts[(c * TPC + t) * K + k + 1]
                    e = s1 - s0
                    base = goff[t, k] * P
                    stream[base:base + e] = src_rel_s[s0:s1]
                    # dstloc entries for this (t,k)
                    dblk = np.full(S[t, k] * P, -1.0, dtype=np.float32)
                    dblk[:e] = d_loc_s[s0:s1]
                    cols = slice(dcol[t] + koff[t, k], dcol[t] + koff[t, k + 1])
                    dstloc[:, cols] = dblk.reshape(-1, P).T
                blk = stream.reshape(-1, 16).T            # [16, n*8]
                cb = idxcols_gk[gi, k]
                idx16[:, cb:cb + n * (P // 16)] = np.tile(blk, (8, 1))
        idx16_all.append(idx16)
        dstloc_all.append(dstloc)
        inv_c = inv_full[c * TPC * P:(c + 1) * TPC * P].reshape(TPC, P).T.copy()
        invcnt_all.append(np.ascontiguousarray(inv_c))     # [128, TPC]
    m.idx16 = idx16_all
    m.dstloc = dstloc_all
    m.invcnt = invcnt_all
    return m


# ----------------------------------------------------------------------------
# Device program
# ----------------------------------------------------------------------------

def build_program(m: Meta, agg_np=ml_dtypes.bfloat16, bufs_gather=2,
                  repeat=1, single_core=False, ablate=()):
    """single_core=True builds a 1-core timing-model variant: no collectives,
    gathers read a pre-cast full feature table (values wrong for layer 2 —
    timing only)."""
    agg_dt = mybir.dt.from_np(np.dtype(agg_np))
    TPC, NPAD, CH, ncores = m.TPC, m.NPAD, m.CH, m.ncores
    S, groups = m.S, m.groups
    NL = TPC * P  # nodes per core

    nc = bacc.Bacc("TRN2", target_bir_lowering=False, debug=False,
                   num_devices=1 if single_core else ncores)

    # I/O
    x_in = nc.dram_tensor("x_local", [NL, F], f32, kind="ExternalInput")
    idx_in = nc.dram_tensor("idx16", [P, max(m.IDXCOLS, 1)], i16,
                            kind="ExternalInput")
    dloc_in = nc.dram_tensor("dstloc", [P, max(m.SLOT_TOT, 1)], f32,
                             kind="ExternalInput")
    inv_in = nc.dram_tensor("invcnt", [P, TPC], f32, kind="ExternalInput")
    wnames = ["wl1t", "wr1t", "wl2t", "wr2t", "wlat", "wlbt"]
    w_in = {n: nc.dram_tensor(n, [F, F], f32, kind="ExternalInput")
            for n in wnames}
    b_in = {n: nc.dram_tensor(n, [F, 1], f32, kind="ExternalInput")
            for n in ["b1", "b2", "blin"]}
    iota_in = nc.dram_tensor("iota", [P, P], f32, kind="ExternalInput")
    y_out = nc.dram_tensor("y_local", [NL, F], f32, kind="ExternalOutput")

    # internal DRAM
    if single_core:
        xa_full = nc.dram_tensor("xa_full_in", [NPAD, F], agg_dt,
                                 kind="ExternalInput")
        xa_sh = None
        h1a_sh = nc.dram_tensor("h1a_sh", [NL, F], agg_dt)
        h1a_full = xa_full
    else:
        xa_sh = nc.dram_tensor("xa_sh", [NL, F], agg_dt)
        xa_full = nc.dram_tensor("xa_full", [NPAD, F], agg_dt,
                                 addr_space="Shared")
        h1a_sh = nc.dram_tensor("h1a_sh", [NL, F], agg_dt)
        h1a_full = nc.dram_tensor("h1a_full", [NPAD, F], agg_dt,
                                  addr_space="Shared")
    h1t_scr = nc.dram_tensor("h1t_scr", [NL, F], f32)

    rg = [list(range(ncores))]
    max_slots_t = int(m.slots_t.max()) if TPC else 1
    max_ngk = max((max(n_k) for n_k in m.n_gk), default=1)
    max_gslots = max((sum(n_k) for n_k in m.n_gk), default=1)

    from contextlib import ExitStack
    with tile.TileContext(nc) as tc, ExitStack() as es:
        cpool = es.enter_context(tc.tile_pool(name="const", bufs=1))
        gpool = es.enter_context(tc.tile_pool(name="gather", bufs=bufs_gather))
        ipool = es.enter_context(tc.tile_pool(name="gidx", bufs=2))
        dpool = es.enter_context(tc.tile_pool(name="dloc", bufs=2))
        opool = es.enter_context(tc.tile_pool(name="onehot", bufs=2))
        wpool = es.enter_context(tc.tile_pool(name="work", bufs=3))
        xpool = es.enter_context(tc.tile_pool(name="xprep", bufs=3))
        pg = es.enter_context(tc.tile_pool(name="pagg", bufs=2, space="PSUM"))
        pt = es.enter_context(tc.tile_pool(name="ptr", bufs=2, space="PSUM"))
        ph = es.enter_context(tc.tile_pool(name="ph", bufs=2, space="PSUM"))

        # constants
        iota_sb = cpool.tile([P, P], f32, tag="iota")
        nc.sync.dma_start(iota_sb[:], iota_in.ap())
        ident = cpool.tile([P, P], f32, tag="ident")
        make_identity(nc, ident[:])
        w_sb = {}
        for n in wnames:
            w_sb[n] = cpool.tile([F, F], f32, tag=n, name=f"w_{n}")
            nc.sync.dma_start(w_sb[n][:], w_in[n].ap())
        b_sb = {}
        for n in ["b1", "b2", "blin"]:
            b_sb[n] = cpool.tile([F, 1], f32, tag=n, name=f"b_{n}")
            nc.sync.dma_start(b_sb[n][:], b_in[n].ap())
        inv_sb = cpool.tile([P, TPC], f32, tag="invcnt")
        nc.sync.dma_start(inv_sb[:], inv_in.ap())

        def phase_a():
            # cast x shard to agg dtype, AllGather
            for t in range(TPC):
                xt = xpool.tile([P, F], f32, tag="xcast", name="xt_a")
                nc.sync.dma_start(xt[:], x_in.ap()[t * P:(t + 1) * P, :])
                if agg_dt == f32:
                    nc.sync.dma_start(xa_sh.ap()[t * P:(t + 1) * P, :], xt[:])
                else:
                    nc.gpsimd.dma_start(xa_sh.ap()[t * P:(t + 1) * P, :],
                                        xt[:])
            nc.gpsimd.collective_compute(
                "AllGather", mybir.AluOpType.bypass, replica_groups=rg,
                ins=[xa_sh.ap()], outs=[xa_full.ap()])

        def layer(src_full, layer_idx):
            """layer_idx 1: x -> h1 (store h1T scratch + h1 rows agg-dt).
            layer_idx 2: h1 -> h2 -> fused final linear -> y rows."""
            wl = w_sb["wl1t" if layer_idx == 1 else "wl2t"]
            wr = w_sb["wr1t" if layer_idx == 1 else "wr2t"]
            bb = b_sb["b1" if layer_idx == 1 else "b2"]
            for gi, gr in enumerate(groups):
                # gathers for this group
                gbufs = []
                for k in range(K):
                    n = m.n_gk[gi][k]
                    if n == 0:
                        gbufs.append(None)
                        continue
                    gb = gpool.tile([P, n, F], agg_dt, tag=f"gbuf{k}")
                    if "gather" not in ablate:
                        it = ipool.tile([P, n * (P // 16)], i16, tag="gidx")
                        cb = m.idxcols_gk[gi, k]
                        nc.sync.dma_start(
                            it[:], idx_in.ap()[:, cb:cb + n * (P // 16)])
                        nc.gpsimd.dma_gather(
                            gb[:], src_full.ap()[k * CH:(k + 1) * CH, :],
                            it[:], n * P, n * P, F, single_packet=False)
                    gbufs.append(gb)
                # dstloc for the group
                c0 = int(m.dcol[gr[0]])
                c1 = int(m.dcol[gr[-1] + 1])
                dl = None
                if c1 > c0:
                    dl = dpool.tile([P, c1 - c0], f32, tag="dloc")
                    nc.sync.dma_start(dl[:], dloc_in.ap()[:, c0:c1])

                for t in gr:
                    nst = int(m.slots_t[t])
                    # one-hot for all slots of tile t
                    if nst > 0:
                        oh = opool.tile([P, nst * P], agg_dt, tag="onehot")
                        if "onehot" not in ablate:
                            dseg = dl[:, int(m.dcol[t]) - c0:
                                      int(m.dcol[t]) - c0 + nst]
                            nc.vector.tensor_tensor(
                                out=oh[:].rearrange("p (s d) -> p s d", d=P),
                                in0=dseg[:, :, None].to_broadcast(
                                    [P, nst, P]),
                                in1=iota_sb[:, None, :].to_broadcast(
                                    [P, nst, P]),
                                op=mybir.AluOpType.is_equal)
                        pagg = pg.tile([P, F], f32, tag="pagg")
                        if "mm" not in ablate:
                            mm, last = 0, nst - 1
                            for k in range(K):
                                gb = gbufs[k]
                                for s in range(int(S[t, k])):
                                    col = int(m.koff[t, k]) + s
                                    slot = int(m.goff[t, k]) + s
                                    nc.tensor.matmul(
                                        pagg[:], oh[:, col * P:(col + 1) * P],
                                        gb[:, slot, :],
                                        start=(mm == 0), stop=(mm == last))
                                    mm += 1
                        else:
                            nc.tensor.matmul(pagg[:], oh[:, :P],
                                             gbufs[0][:, 0, :] if gbufs[0]
                                             is not None else oh[:, :P],
                                             start=True, stop=True)
                        aggm = wpool.tile([P, F], f32, tag="aggm")
                        nc.vector.tensor_scalar_mul(
                            aggm[:], pagg[:], inv_sb[:, t:t + 1])
                    else:
                        aggm = wpool.tile([P, F], f32, tag="aggm")
                        nc.vector.memset(aggm[:], 0.0)
                    ptr1 = pt.tile([P, F], f32, tag="ptr")
                    nc.tensor.transpose(ptr1[:], aggm[:], ident[:])
                    aggT = wpool.tile([P, F], f32, tag="aggT")
                    nc.scalar.copy(aggT[:], ptr1[:])

                    # second operand: xT (layer1) or h1T scratch (layer2)
                    if layer_idx == 1:
                        xt = xpool.tile([P, F], f32, tag="xl")
                        nc.sync.dma_start(xt[:], x_in.ap()[t * P:(t + 1) * P, :])
                        ptr2 = pt.tile([P, F], f32, tag="ptr")
                        nc.tensor.transpose(ptr2[:], xt[:], ident[:])
                        rT = wpool.tile([P, F], f32, tag="rT")
                        nc.scalar.copy(rT[:], ptr2[:])
                    else:
                        rT = wpool.tile([P, F], f32, tag="rT")
                        nc.sync.dma_start(rT[:],
                                          h1t_scr.ap()[t * P:(t + 1) * P, :])

                    phh = ph.tile([P, F], f32, tag="ph")
                    nc.tensor.matmul(phh[:], wl[:], aggT[:],
                                     start=True, stop=False)
                    nc.tensor.matmul(phh[:], wr[:], rT[:],
                                     start=False, stop=True)
                    hT = wpool.tile([P, F], f32, tag="hT")
                    nc.scalar.activation(hT[:], phh[:],
                                         mybir.ActivationFunctionType.Relu,
                                         bias=bb[:, :1])

                    if layer_idx == 1:
                        # store h1T scratch + h1 rows (agg dtype) for AllGather
                        nc.sync.dma_start(h1t_scr.ap()[t * P:(t + 1) * P, :],
                                          hT[:])
                        ptr3 = pt.tile([P, F], f32, tag="ptr")
                        nc.tensor.transpose(ptr3[:], hT[:], ident[:])
                        rows = wpool.tile([P, F], agg_dt, tag="rows")
                        nc.vector.tensor_copy(rows[:], ptr3[:])
                        if agg_dt == f32:
                            nc.sync.dma_start(
                                h1a_sh.ap()[t * P:(t + 1) * P, :], rows[:])
                        else:
                            nc.gpsimd.dma_start(
                                h1a_sh.ap()[t * P:(t + 1) * P, :], rows[:])
                    else:
                        # fused final: [h1 h2] @ Wlin.T + blin, relu
                        pout = ph.tile([P, F], f32, tag="ph")
                        nc.tensor.matmul(pout[:], w_sb["wlat"][:], rT[:],
                                         start=True, stop=False)
                        nc.tensor.matmul(pout[:], w_sb["wlbt"][:], hT[:],
                                         start=False, stop=True)
                        oT = wpool.tile([P, F], f32, tag="oT")
                        nc.scalar.activation(
                            oT[:], pout[:], mybir.ActivationFunctionType.Relu,
                            bias=b_sb["blin"][:, :1])
                        ptr4 = pt.tile([P, F], f32, tag="ptr")
                        nc.tensor.transpose(ptr4[:], oT[:], ident[:])
                        orow = wpool.tile([P, F], f32, tag="orow")
                        nc.scalar.copy(orow[:], ptr4[:])
                        nc.sync.dma_start(y_out.ap()[t * P:(t + 1) * P, :],
                                          orow[:])

        for _rep in range(repeat):
            if not single_core:
                phase_a()
            layer(xa_full, 1)
            if not single_core:
                nc.gpsimd.collective_compute(
                    "AllGather", mybir.AluOpType.bypass, replica_groups=rg,
                    ins=[h1a_sh.ap()], outs=[h1a_full.ap()])
            layer(h1a_full, 2)

    nc.compile()
    return nc


# ----------------------------------------------------------------------------
# Full pipeline
# ----------------------------------------------------------------------------

def make_in_maps(m: Meta, inputs: dict):
    """Per-core input dicts from full inputs + meta."""
    x = np.asarray(inputs["x"], dtype=np.float32)
    TPC, ncores = m.TPC, m.ncores
    NL = TPC * P
    xpad = np.zeros((m.NPAD, F), dtype=np.float32)
    xpad[:m.N] = x
    iota = np.broadcast_to(np.arange(P, dtype=np.float32), (P, P)).copy()
    base = {
        "wl1t": np.ascontiguousarray(np.asarray(inputs["Wl1"], np.float32).T),
        "wr1t": np.ascontiguousarray(np.asarray(inputs["Wr1"], np.float32).T),
        "wl2t": np.ascontiguousarray(np.asarray(inputs["Wl2"], np.float32).T),
        "wr2t": np.ascontiguousarray(np.asarray(inputs["Wr2"], np.float32).T),
        "wlat": np.ascontiguousarray(
            np.asarray(inputs["Wlin"], np.float32)[:, :F].T),
        "wlbt": np.ascontiguousarray(
            np.asarray(inputs["Wlin"], np.float32)[:, F:].T),
        "b1": np.asarray(inputs["b1"], np.float32).reshape(F, 1),
        "b2": np.asarray(inputs["b2"], np.float32).reshape(F, 1),
        "blin": np.asarray(inputs["blin"], np.float32).reshape(F, 1),
        "iota": iota,
    }
    maps = []
    for c in range(ncores):
        d = dict(base)
        d["x_local"] = np.ascontiguousarray(xpad[c * NL:(c + 1) * NL])
        d["idx16"] = m.idx16[c]
        d["dstloc"] = m.dstloc[c]
        d["invcnt"] = m.invcnt[c]
        maps.append(d)
    return maps


def assemble_output(m: Meta, results):
    ys = [results[c]["y_local"] for c in range(m.ncores)]
    return np.concatenate(ys, axis=0)[:m.N].astype(np.float32)


# ----------------------------------------------------------------------------
# kernel() entry point (appended to gnn_bass source to form kernel.py)
# ----------------------------------------------------------------------------

_N = 100000
_NCORES = 8
_GT = 7
_AGG = ml_dtypes.bfloat16

_cache = {}


def _get_program(edge_key, edge_index):
    if edge_key not in _cache:
        m = preprocess(edge_index, _N, _NCORES, _GT)
        nc = build_program(m, agg_np=_AGG)
        _cache[edge_key] = (m, nc)
    return _cache[edge_key]


def kernel(**inputs):
    from concourse.bass_utils import run_bass_kernel_spmd
    edge_index = np.asarray(inputs["edge_index"])
    assert edge_index.shape == (2, 1600000), edge_index.shape
    assert np.asarray(inputs["x"]).shape == (_N, 128)
    key = hash(edge_index.tobytes())
    m, nc = _get_program(key, edge_index)
    in_maps = make_in_maps(m, inputs)
    res = run_bass_kernel_spmd(nc, in_maps, list(range(_NCORES)))
    return assemble_output(m, [res.results[c] for c in range(_NCORES)])



# revision 14
# speedup vs baseline: 10.2970x; 10.2970x over previous
"""GraphSAGE x2 + JumpingKnowledge(cat) + Linear block on TRN2, 8-core SPMD.

Host preprocessing shards destination nodes across cores, groups/pads edges,
and builds int16 gather indices; the device kernel does all O(E*F) work:
per-edge feature gather (SWDGE dma_gather on 4 queues), segment-sum via
one-hot matmuls into PSUM, SAGE projections (bf16 operands, f32 accum),
and the final JK linear. xT and h1T live in SBUF; no DRAM scratch.
"""

import math
import numpy as np
import ml_dtypes

import concourse.bass as bass
import concourse.bacc as bacc
import concourse.tile as tile
from concourse import mybir
from concourse.masks import make_identity

P = 128          # partitions / tile height
F = 128          # feature dim (all layers)
K = 4            # number of src chunks (int16 index limit)

f32 = mybir.dt.float32
bf16 = mybir.dt.bfloat16
i16 = mybir.dt.int16


# ----------------------------------------------------------------------------
# Host preprocessing
# ----------------------------------------------------------------------------

class Meta:
    pass


def preprocess(edge_index: np.ndarray, N: int, ncores: int, group_tiles: int):
    """Build per-core padded edge streams with an SPMD-uniform structure.

    Returns Meta with:
      TPC, NPAD, CH, S [TPC,K], groups (list of tile ranges), per-(g,k) slot
      counts, idx16 [ncores][128, IDXCOLS], dstloc [ncores][128, SLOT_TOT],
      invcnt [ncores][128, TPC], col/offset tables.
    """
    m = Meta()
    TPC = math.ceil(N / (ncores * P))
    NPAD = ncores * TPC * P
    CH = math.ceil(NPAD / (K * P)) * P
    assert CH <= 32768
    m.N, m.ncores, m.TPC, m.NPAD, m.CH = N, ncores, TPC, NPAD, CH

    src = edge_index[0].astype(np.int64)
    dst = edge_index[1].astype(np.int64)
    E = src.shape[0]

    core = dst // (TPC * P)
    t_loc = (dst % (TPC * P)) // P
    d_loc = dst % P
    kk = src // CH
    src_rel = (src - kk * CH).astype(np.int16)

    key = ((core * TPC + t_loc) * K + kk).astype(np.int64)
    order = np.argsort(key, kind="stable")
    src_rel_s = src_rel[order]
    d_loc_s = d_loc[order].astype(np.float32)
    counts = np.bincount(key, minlength=ncores * TPC * K).reshape(ncores, TPC, K)
    starts = np.zeros(ncores * TPC * K + 1, dtype=np.int64)
    np.cumsum(counts.reshape(-1), out=starts[1:])

    S = np.ceil(counts / P).astype(np.int64).max(axis=0)      # [TPC, K]
    m.S = S
    slots_t = S.sum(axis=1)                                    # [TPC]
    m.slots_t = slots_t
    m.SLOT_TOT = int(S.sum())

    # groups of tiles
    groups = [list(range(g, min(g + group_tiles, TPC)))
              for g in range(0, TPC, group_tiles)]
    m.groups = groups
    m.n_gk = [[int(S[g, k].sum()) for k in range(K)] for g in
              [np.array(gr) for gr in groups]]                 # slots per (g,k)

    # column offsets
    # dstloc: tile t occupies cols [dcol[t], dcol[t]+slots_t[t])
    dcol = np.zeros(TPC + 1, dtype=np.int64)
    np.cumsum(slots_t, out=dcol[1:])
    m.dcol = dcol
    # within tile t, chunk k starts at slot koff[t,k]
    koff = np.zeros((TPC, K + 1), dtype=np.int64)
    np.cumsum(S, axis=1, out=koff[:, 1:])
    m.koff = koff
    # gather stream: per (g,k), tile t at slot goff[t,k] (within that gather)
    goff = np.zeros((TPC, K), dtype=np.int64)
    for gi, gr in enumerate(groups):
        for k in range(K):
            off = 0
            for t in gr:
                goff[t, k] = off
                off += S[t, k]
    m.goff = goff
    # idx16 col offsets per (g,k): [128, n_gk*8] each
    idxcol = np.zeros((len(groups), K + 1), dtype=np.int64)
    flat = 0
    idxcols_gk = np.zeros((len(groups), K), dtype=np.int64)
    for gi in range(len(groups)):
        for k in range(K):
            idxcols_gk[gi, k] = flat
            flat += m.n_gk[gi][k] * (P // 16)
    m.idxcols_gk = idxcols_gk
    m.IDXCOLS = int(flat)

    # per-core data
    idx16_all, dstloc_all, invcnt_all = [], [], []
    cnt_full = np.bincount(dst, minlength=NPAD).astype(np.float32)
    inv_full = 1.0 / np.maximum(cnt_full, 1.0)
    for c in range(ncores):
        # gather idx stream, per (g,k)
        idx16 = np.zeros((P, m.IDXCOLS), dtype=np.int16)
        dstloc = np.full((P, m.SLOT_TOT), -1.0, dtype=np.float32)
        for gi, gr in enumerate(groups):
            for k in range(K):
                n = m.n_gk[gi][k]
                if n == 0:
                    continue
                stream = np.zeros(n * P, dtype=np.int16)
                for t in gr:
                    s0 = starts[(c * TPC + t) * K + k]
                    s1 = starts[(c * TPC + t) * K + k + 1]
                    e = s1 - s0
                    base = goff[t, k] * P
                    stream[base:base + e] = src_rel_s[s0:s1]
                    # dstloc entries for this (t,k)
                    dblk = np.full(S[t, k] * P, -1.0, dtype=np.float32)
                    dblk[:e] = d_loc_s[s0:s1]
                    cols = slice(dcol[t] + koff[t, k], dcol[t] + koff[t, k + 1])
                    dstloc[:, cols] = dblk.reshape(-1, P).T
                blk = stream.reshape(-1, 16).T            # [16, n*8]
                cb = idxcols_gk[gi, k]
                idx16[:, cb:cb + n * (P // 16)] = np.tile(blk, (8, 1))
        idx16_all.append(idx16)
        dstloc_all.append(dstloc)
        inv_c = inv_full[c * TPC * P:(c + 1) * TPC * P].reshape(TPC, P).T.copy()
        invcnt_all.append(np.ascontiguousarray(inv_c))     # [128, TPC]
    m.idx16 = idx16_all
    m.dstloc = dstloc_all
    m.invcnt = invcnt_all
    return m


# ----------------------------------------------------------------------------
# Device program
# ----------------------------------------------------------------------------

def build_program(m: Meta, agg_np=ml_dtypes.bfloat16, bufs_gather=2,
                  repeat=1, single_core=False, ablate=()):
    """single_core=True builds a 1-core timing-model variant: no collectives,
    gathers read a pre-cast full feature table (values wrong for layer 2 —
    timing only)."""
    agg_dt = mybir.dt.from_np(np.dtype(agg_np))
    assert agg_dt == bf16
    TPC, NPAD, CH, ncores = m.TPC, m.NPAD, m.CH, m.ncores
    S, groups = m.S, m.groups
    NL = TPC * P  # nodes per core
    GT = max(len(gr) for gr in groups)

    nc = bacc.Bacc("TRN2", target_bir_lowering=False, debug=False,
                   num_devices=1 if single_core else ncores,
                   num_swdge_queues=4)

    # I/O  (x supplied twice from host: bf16 rows for the AllGather and
    # bf16 feature-major xT for the self term)
    xr_in = nc.dram_tensor("x_rows", [NL, F], bf16, kind="ExternalInput")
    xt_in = nc.dram_tensor("x_t", [P, NL], bf16, kind="ExternalInput")
    idx_in = nc.dram_tensor("idx16", [P, max(m.IDXCOLS, 1)], i16,
                            kind="ExternalInput")
    dloc_in = nc.dram_tensor("dstloc", [P, max(m.SLOT_TOT, 1)], f32,
                             kind="ExternalInput")
    inv_in = nc.dram_tensor("invcnt", [P, TPC], f32, kind="ExternalInput")
    wnames = ["wl1t", "wr1t", "wl2t", "wr2t", "wlat", "wlbt"]
    w_in = {n: nc.dram_tensor(n, [F, F], bf16, kind="ExternalInput")
            for n in wnames}
    b_in = {n: nc.dram_tensor(n, [F, 1], f32, kind="ExternalInput")
            for n in ["b1", "b2", "blin"]}
    iota_in = nc.dram_tensor("iota", [P, P], f32, kind="ExternalInput")
    y_out = nc.dram_tensor("y_local", [NL, F], f32, kind="ExternalOutput")

    # internal DRAM
    if single_core:
        xa_full = nc.dram_tensor("xa_full_in", [NPAD, F], bf16,
                                 kind="ExternalInput")
        xa_sh = None
        h1a_sh = nc.dram_tensor("h1a_sh", [NL, F], bf16)
        h1a_full = xa_full
    else:
        xa_sh = nc.dram_tensor("xa_sh", [NL, F], bf16)
        xa_full = nc.dram_tensor("xa_full", [NPAD, F], bf16,
                                 addr_space="Shared")
        h1a_sh = nc.dram_tensor("h1a_sh", [NL, F], bf16)
        h1a_full = nc.dram_tensor("h1a_full", [NPAD, F], bf16,
                                  addr_space="Shared")

    rg = [list(range(ncores))]

    from contextlib import ExitStack
    with tile.TileContext(nc) as tc, ExitStack() as es:
        cpool = es.enter_context(tc.tile_pool(name="const", bufs=1))
        gpool = es.enter_context(tc.tile_pool(name="gather", bufs=bufs_gather))
        ipool = es.enter_context(tc.tile_pool(name="gidx", bufs=8))
        dpool = es.enter_context(tc.tile_pool(name="dloc", bufs=3))
        opool = es.enter_context(tc.tile_pool(name="onehot", bufs=2))
        wpool = es.enter_context(tc.tile_pool(name="work", bufs=3))
        ypool = es.enter_context(tc.tile_pool(name="yrow", bufs=2))
        pg = es.enter_context(tc.tile_pool(name="pagg", bufs=2, space="PSUM"))
        pt = es.enter_context(tc.tile_pool(name="ptr", bufs=2, space="PSUM"))
        ph = es.enter_context(tc.tile_pool(name="ph", bufs=2, space="PSUM"))

        # constants
        iota_sb = cpool.tile([P, P], f32, tag="iota")
        nc.sync.dma_start(iota_sb[:], iota_in.ap())
        ident = cpool.tile([P, P], bf16, tag="ident")
        make_identity(nc, ident[:])
        w_sb = {}
        for n in wnames:
            w_sb[n] = cpool.tile([F, F], bf16, tag=n, name=f"w_{n}")
            nc.sync.dma_start(w_sb[n][:], w_in[n].ap())
        b_sb = {}
        for n in ["b1", "b2", "blin"]:
            b_sb[n] = cpool.tile([F, 1], f32, tag=n, name=f"b_{n}")
            nc.sync.dma_start(b_sb[n][:], b_in[n].ap())
        inv_sb = cpool.tile([P, TPC], f32, tag="invcnt")
        nc.sync.dma_start(inv_sb[:], inv_in.ap())
        # persistent feature-major buffers
        xT_sb = cpool.tile([P, NL], bf16, tag="xT")
        nc.sync.dma_start(xT_sb[:], xt_in.ap())
        h1T_sb = cpool.tile([P, NL], bf16, tag="h1T")

        def layer(src_full, layer_idx):
            """layer_idx 1: x -> h1 (h1T into SBUF + h1 rows for AllGather).
            layer_idx 2: h1 -> h2 -> fused final linear -> y rows."""
            wl = w_sb["wl1t" if layer_idx == 1 else "wl2t"]
            wr = w_sb["wr1t" if layer_idx == 1 else "wr2t"]
            bb = b_sb["b1" if layer_idx == 1 else "b2"]
            rT_all = xT_sb if layer_idx == 1 else h1T_sb
            for gi, gr in enumerate(groups):
                # gathers for this group
                gbufs = []
                for k in range(K):
                    n = m.n_gk[gi][k]
                    if n == 0:
                        gbufs.append(None)
                        continue
                    gb = gpool.tile([P, n, F], bf16, tag=f"gbuf{k}")
                    if "lineardma" in ablate:
                        nc.sync.dma_start(
                            gb[:].rearrange("p s f -> p (s f)"),
                            src_full.ap()[:128 * n, :].rearrange(
                                "(p b) f -> p (b f)", p=128))
                    elif "gather" in ablate:
                        nc.vector.memset(gb[:, :1, :1], 0.0)
                    else:
                        it = ipool.tile([P, n * (P // 16)], i16, tag="gidx")
                        cb = m.idxcols_gk[gi, k]
                        nc.sync.dma_start(
                            it[:], idx_in.ap()[:, cb:cb + n * (P // 16)])
                        nc.gpsimd.dma_gather(
                            gb[:], src_full.ap()[k * CH:(k + 1) * CH, :],
                            it[:], n * P, n * P, F, single_packet=False,
                            queue_num=k)
                    gbufs.append(gb)
                # dstloc for the group
                c0 = int(m.dcol[gr[0]])
                c1 = int(m.dcol[gr[-1] + 1])
                dl = None
                if c1 > c0:
                    dl = dpool.tile([P, c1 - c0], f32, tag="dloc")
                    nc.sync.dma_start(dl[:], dloc_in.ap()[:, c0:c1])
                # group-batched output buffers
                ng = len(gr)
                if layer_idx == 1:
                    hr_g = ypool.tile([P, ng, F], bf16, tag="h1rows")
                else:
                    yg = ypool.tile([P, ng, F], f32, tag="yrows")

                for ti, t in enumerate(gr):
                    rT = rT_all[:, t * P:(t + 1) * P]
                    # self/JK terms first: no gather dependency, keeps PE busy
                    phh = ph.tile([P, F], f32, tag="ph")
                    nc.tensor.matmul(phh[:], wr[:], rT,
                                     start=True, stop=False)
                    if layer_idx == 2:
                        pout = ph.tile([P, F], f32, tag="ph2")
                        nc.tensor.matmul(pout[:], w_sb["wlat"][:], rT,
                                         start=True, stop=False)
                    nst = int(m.slots_t[t])
                    # one-hot for all slots of tile t
                    if nst > 0:
                        oh = opool.tile([P, nst * P], bf16, tag="onehot")
                        if "onehot" in ablate:
                            nc.vector.memset(oh[:, :1], 0.0)
                        else:
                            dseg = dl[:, int(m.dcol[t]) - c0:
                                      int(m.dcol[t]) - c0 + nst]
                            nc.vector.tensor_tensor(
                                out=oh[:].rearrange("p (s d) -> p s d", d=P),
                                in0=dseg[:, :, None].to_broadcast(
                                    [P, nst, P]),
                                in1=iota_sb[:, None, :].to_broadcast(
                                    [P, nst, P]),
                                op=mybir.AluOpType.is_equal)
                        pagg = pg.tile([P, F], f32, tag="pagg")
                        if "mm" not in ablate:
                            mm, last = 0, nst - 1
                            for k in range(K):
                                gb = gbufs[k]
                                for s in range(int(S[t, k])):
                                    col = int(m.koff[t, k]) + s
                                    slot = int(m.goff[t, k]) + s
                                    nc.tensor.matmul(
                                        pagg[:], oh[:, col * P:(col + 1) * P],
                                        gb[:, slot, :],
                                        start=(mm == 0), stop=(mm == last))
                                    mm += 1
                        else:
                            nc.tensor.matmul(pagg[:], oh[:, :P],
                                             gbufs[0][:, 0, :] if gbufs[0]
                                             is not None else oh[:, :P],
                                             start=True, stop=True)
                        aggm = wpool.tile([P, F], bf16, tag="aggm")
                        nc.vector.tensor_scalar_mul(
                            aggm[:], pagg[:], inv_sb[:, t:t + 1])
                    else:
                        aggm = wpool.tile([P, F], bf16, tag="aggm")
                        nc.vector.memset(aggm[:], 0.0)
                    ptr1 = pt.tile([P, F], bf16, tag="ptr")
                    nc.tensor.transpose(ptr1[:], aggm[:], ident[:])
                    aggT = wpool.tile([P, F], bf16, tag="aggT")
                    nc.scalar.copy(aggT[:], ptr1[:])
                    nc.tensor.matmul(phh[:], wl[:], aggT[:],
                                     start=False, stop=True)

                    if layer_idx == 1:
                        # h1T into persistent SBUF; h1 rows (bf16) for gather
                        hT = h1T_sb[:, t * P:(t + 1) * P]
                        nc.scalar.activation(
                            hT, phh[:], mybir.ActivationFunctionType.Relu,
                            bias=bb[:, :1])
                        ptr3 = pt.tile([P, F], bf16, tag="ptr")
                        nc.tensor.transpose(ptr3[:], hT, ident[:])
                        nc.scalar.copy(hr_g[:, ti, :], ptr3[:])
                    else:
                        hT = wpool.tile([P, F], bf16, tag="hT")
                        nc.scalar.activation(
                            hT[:], phh[:], mybir.ActivationFunctionType.Relu,
                            bias=bb[:, :1])
                        # fused final: [h1 h2] @ Wlin.T + blin, relu
                        nc.tensor.matmul(pout[:], w_sb["wlbt"][:], hT[:],
                                         start=False, stop=True)
                        oT = wpool.tile([P, F], bf16, tag="oT")
                        nc.scalar.activation(
                            oT[:], pout[:], mybir.ActivationFunctionType.Relu,
                            bias=b_sb["blin"][:, :1])
                        ptr4 = pt.tile([P, F], bf16, tag="ptr")
                        nc.tensor.transpose(ptr4[:], oT[:], ident[:])
                        nc.scalar.copy(yg[:, ti, :], ptr4[:])

                # group-batched stores
                if layer_idx == 1:
                    nc.sync.dma_start(
                        h1a_sh.ap()[gr[0] * P:(gr[-1] + 1) * P, :].rearrange(
                            "(t p) f -> p t f", p=P),
                        hr_g[:])
                else:
                    nc.sync.dma_start(
                        y_out.ap()[gr[0] * P:(gr[-1] + 1) * P, :].rearrange(
                            "(t p) f -> p t f", p=P),
                        yg[:])

        for _rep in range(repeat):
            if not single_core and "coll" not in ablate:
                nc.sync.dma_start(xa_sh.ap(), xr_in.ap())
                nc.gpsimd.collective_compute(
                    "AllGather", mybir.AluOpType.bypass, replica_groups=rg,
                    ins=[xa_sh.ap()], outs=[xa_full.ap()])
            layer(xa_full, 1)
            if not single_core and "coll" not in ablate:
                nc.gpsimd.collective_compute(
                    "AllGather", mybir.AluOpType.bypass, replica_groups=rg,
                    ins=[h1a_sh.ap()], outs=[h1a_full.ap()])
            layer(h1a_full, 2)

    nc.compile()
    return nc


# ----------------------------------------------------------------------------
# Full pipeline
# ----------------------------------------------------------------------------

def make_in_maps(m: Meta, inputs: dict):
    """Per-core input dicts from full inputs + meta."""
    x = np.asarray(inputs["x"], dtype=np.float32)
    TPC, ncores = m.TPC, m.ncores
    NL = TPC * P
    xpad = np.zeros((m.NPAD, F), dtype=np.float32)
    xpad[:m.N] = x
    xpad_bf = xpad.astype(ml_dtypes.bfloat16)
    iota = np.broadcast_to(np.arange(P, dtype=np.float32), (P, P)).copy()

    def wt(a):
        return np.ascontiguousarray(
            np.asarray(a, np.float32).T.astype(ml_dtypes.bfloat16))

    base = {
        "wl1t": wt(inputs["Wl1"]),
        "wr1t": wt(inputs["Wr1"]),
        "wl2t": wt(inputs["Wl2"]),
        "wr2t": wt(inputs["Wr2"]),
        "wlat": wt(np.asarray(inputs["Wlin"], np.float32)[:, :F]),
        "wlbt": wt(np.asarray(inputs["Wlin"], np.float32)[:, F:]),
        "b1": np.asarray(inputs["b1"], np.float32).reshape(F, 1),
        "b2": np.asarray(inputs["b2"], np.float32).reshape(F, 1),
        "blin": np.asarray(inputs["blin"], np.float32).reshape(F, 1),
        "iota": iota,
    }
    maps = []
    for c in range(ncores):
        d = dict(base)
        xs = xpad_bf[c * NL:(c + 1) * NL]
        d["x_rows"] = np.ascontiguousarray(xs)
        d["x_t"] = np.ascontiguousarray(xs.T)
        d["idx16"] = m.idx16[c]
        d["dstloc"] = m.dstloc[c]
        d["invcnt"] = m.invcnt[c]
        maps.append(d)
    return maps


def assemble_output(m: Meta, results):
    ys = [results[c]["y_local"] for c in range(m.ncores)]
    return np.concatenate(ys, axis=0)[:m.N].astype(np.float32)


# ----------------------------------------------------------------------------
# kernel() entry point
# ----------------------------------------------------------------------------

_N = 100000
_NCORES = 8
_GT = 7
_AGG = ml_dtypes.bfloat16

_cache = {}


def _get_program(edge_key, edge_index):
    if edge_key not in _cache:
        m = preprocess(edge_index, _N, _NCORES, _GT)
        nc = build_program(m, agg_np=_AGG)
        _cache[edge_key] = (m, nc)
    return _cache[edge_key]


def kernel(**inputs):
    from concourse.bass_utils import run_bass_kernel_spmd
    edge_index = np.asarray(inputs["edge_index"])
    assert edge_index.shape == (2, 1600000), edge_index.shape
    assert np.asarray(inputs["x"]).shape == (_N, 128)
    key = hash(edge_index.tobytes())
    m, nc = _get_program(key, edge_index)
    in_maps = make_in_maps(m, inputs)
    res = run_bass_kernel_spmd(nc, in_maps, list(range(_NCORES)))
    return assemble_output(m, [res.results[c] for c in range(_NCORES)])


# revision 18
# speedup vs baseline: 11.4874x; 1.1156x over previous
"""GraphSAGE x2 + JumpingKnowledge(cat) + Linear block on TRN2, 8-core SPMD.

Host preprocessing shards destination nodes across cores, groups/pads edges,
and builds int16 gather indices; the device kernel does all O(E*F) work:
per-edge feature gather (SWDGE dma_gather on 4 queues), segment-sum via
one-hot matmuls into PSUM, SAGE projections (bf16 operands, f32 accum),
and the final JK linear. xT and h1T live in SBUF; no DRAM scratch.
"""

import math
import numpy as np
import ml_dtypes

import concourse.bass as bass
import concourse.bacc as bacc
import concourse.tile as tile
from concourse import mybir
from concourse.masks import make_identity

P = 128          # partitions / tile height
F = 128          # feature dim (all layers)
K = 4            # number of src chunks (int16 index limit)

f32 = mybir.dt.float32
bf16 = mybir.dt.bfloat16
i16 = mybir.dt.int16


# ----------------------------------------------------------------------------
# Host preprocessing
# ----------------------------------------------------------------------------

class Meta:
    pass


def preprocess(edge_index: np.ndarray, N: int, ncores: int, group_tiles: int):
    """Build per-core padded edge streams with an SPMD-uniform structure.

    Returns Meta with:
      TPC, NPAD, CH, S [TPC,K], groups (list of tile ranges), per-(g,k) slot
      counts, idx16 [ncores][128, IDXCOLS], dstloc [ncores][128, SLOT_TOT],
      invcnt [ncores][128, TPC], col/offset tables.
    """
    m = Meta()
    TPC = math.ceil(N / (ncores * P))
    NPAD = ncores * TPC * P
    CH = math.ceil(NPAD / (K * P)) * P
    assert CH <= 32768
    m.N, m.ncores, m.TPC, m.NPAD, m.CH = N, ncores, TPC, NPAD, CH

    src = edge_index[0].astype(np.int64)
    dst = edge_index[1].astype(np.int64)
    E = src.shape[0]

    core = dst // (TPC * P)
    t_loc = (dst % (TPC * P)) // P
    d_loc = dst % P
    kk = src // CH
    src_rel = (src - kk * CH).astype(np.int16)

    key = ((core * TPC + t_loc) * K + kk).astype(np.int64)
    order = np.argsort(key, kind="stable")
    src_rel_s = src_rel[order]
    d_loc_s = d_loc[order].astype(np.float32)
    counts = np.bincount(key, minlength=ncores * TPC * K).reshape(ncores, TPC, K)
    starts = np.zeros(ncores * TPC * K + 1, dtype=np.int64)
    np.cumsum(counts.reshape(-1), out=starts[1:])

    S = np.ceil(counts / P).astype(np.int64).max(axis=0)      # [TPC, K]
    m.S = S
    slots_t = S.sum(axis=1)                                    # [TPC]
    m.slots_t = slots_t
    m.SLOT_TOT = int(S.sum())

    # groups of tiles
    groups = [list(range(g, min(g + group_tiles, TPC)))
              for g in range(0, TPC, group_tiles)]
    m.groups = groups
    m.n_gk = [[int(S[g, k].sum()) for k in range(K)] for g in
              [np.array(gr) for gr in groups]]                 # slots per (g,k)

    # column offsets
    # dstloc: tile t occupies cols [dcol[t], dcol[t]+slots_t[t])
    dcol = np.zeros(TPC + 1, dtype=np.int64)
    np.cumsum(slots_t, out=dcol[1:])
    m.dcol = dcol
    # within tile t, chunk k starts at slot koff[t,k]
    koff = np.zeros((TPC, K + 1), dtype=np.int64)
    np.cumsum(S, axis=1, out=koff[:, 1:])
    m.koff = koff
    # gather stream: per (g,k), tile t at slot goff[t,k] (within that gather)
    goff = np.zeros((TPC, K), dtype=np.int64)
    for gi, gr in enumerate(groups):
        for k in range(K):
            off = 0
            for t in gr:
                goff[t, k] = off
                off += S[t, k]
    m.goff = goff
    # idx16 col offsets per (g,k): [128, n_gk*8] each
    idxcol = np.zeros((len(groups), K + 1), dtype=np.int64)
    flat = 0
    idxcols_gk = np.zeros((len(groups), K), dtype=np.int64)
    for gi in range(len(groups)):
        for k in range(K):
            idxcols_gk[gi, k] = flat
            flat += m.n_gk[gi][k] * (P // 16)
    m.idxcols_gk = idxcols_gk
    m.IDXCOLS = int(flat)

    # per-core data
    idx16_all, dstloc_all, invcnt_all = [], [], []
    cnt_full = np.bincount(dst, minlength=NPAD).astype(np.float32)
    inv_full = 1.0 / np.maximum(cnt_full, 1.0)
    for c in range(ncores):
        # gather idx stream, per (g,k)
        idx16 = np.zeros((P, m.IDXCOLS), dtype=np.int16)
        dstloc = np.full((P, m.SLOT_TOT), -1.0, dtype=np.float32)
        for gi, gr in enumerate(groups):
            for k in range(K):
                n = m.n_gk[gi][k]
                if n == 0:
                    continue
                stream = np.zeros(n * P, dtype=np.int16)
                for t in gr:
                    s0 = starts[(c * TPC + t) * K + k]
                    s1 = starts[(c * TPC + t) * K + k + 1]
                    e = s1 - s0
                    base = goff[t, k] * P
                    stream[base:base + e] = src_rel_s[s0:s1]
                    # dstloc entries for this (t,k)
                    dblk = np.full(S[t, k] * P, -1.0, dtype=np.float32)
                    dblk[:e] = d_loc_s[s0:s1]
                    cols = slice(dcol[t] + koff[t, k], dcol[t] + koff[t, k + 1])
                    dstloc[:, cols] = dblk.reshape(-1, P).T
                blk = stream.reshape(-1, 16).T            # [16, n*8]
                cb = idxcols_gk[gi, k]
                idx16[:, cb:cb + n * (P // 16)] = np.tile(blk, (8, 1))
        idx16_all.append(idx16)
        dstloc_all.append(dstloc)
        inv_c = inv_full[c * TPC * P:(c + 1) * TPC * P].reshape(TPC, P).T.copy()
        invcnt_all.append(np.ascontiguousarray(inv_c))     # [128, TPC]
    m.idx16 = idx16_all
    m.dstloc = dstloc_all
    m.invcnt = invcnt_all
    return m


# ----------------------------------------------------------------------------
# Device program
# ----------------------------------------------------------------------------

def build_program(m: Meta, agg_np=ml_dtypes.bfloat16, bufs_gather=2,
                  repeat=1, single_core=False, ablate=()):
    """single_core=True builds a 1-core timing-model variant: no collectives,
    gathers read a pre-cast full feature table (values wrong for layer 2 —
    timing only)."""
    agg_dt = mybir.dt.from_np(np.dtype(agg_np))
    assert agg_dt == bf16
    TPC, NPAD, CH, ncores = m.TPC, m.NPAD, m.CH, m.ncores
    S, groups = m.S, m.groups
    NL = TPC * P  # nodes per core
    GT = max(len(gr) for gr in groups)

    nc = bacc.Bacc("TRN2", target_bir_lowering=False, debug=False,
                   num_devices=1 if single_core else ncores,
                   num_swdge_queues=4)

    # I/O  (x supplied twice from host: bf16 rows for the AllGather and
    # bf16 feature-major xT for the self term)
    xr_in = nc.dram_tensor("x_rows", [NL, F], bf16, kind="ExternalInput")
    xt_in = nc.dram_tensor("x_t", [P, NL], bf16, kind="ExternalInput")
    idx_in = nc.dram_tensor("idx16", [P, max(m.IDXCOLS, 1)], i16,
                            kind="ExternalInput")
    dloc_in = nc.dram_tensor("dstloc", [P, max(m.SLOT_TOT, 1)], f32,
                             kind="ExternalInput")
    inv_in = nc.dram_tensor("invcnt", [P, TPC], f32, kind="ExternalInput")
    wnames = ["wl1t", "wr1t", "wl2t", "wr2t", "wlat", "wlbt"]
    w_in = {n: nc.dram_tensor(n, [F, F], bf16, kind="ExternalInput")
            for n in wnames}
    b_in = {n: nc.dram_tensor(n, [F, 1], f32, kind="ExternalInput")
            for n in ["b1", "b2", "blin"]}
    iota_in = nc.dram_tensor("iota", [P, P], f32, kind="ExternalInput")
    y_out = nc.dram_tensor("y_local", [NL, F], f32, kind="ExternalOutput")

    # internal DRAM
    if single_core:
        xa_full = nc.dram_tensor("xa_full_in", [NPAD, F], bf16,
                                 kind="ExternalInput")
        xa_sh = None
        h1a_sh = nc.dram_tensor("h1a_sh", [NL, F], bf16)
        h1a_full = xa_full
    else:
        xa_sh = nc.dram_tensor("xa_sh", [NL, F], bf16)
        xa_full = nc.dram_tensor("xa_full", [NPAD, F], bf16,
                                 addr_space="Shared")
        h1a_sh = nc.dram_tensor("h1a_sh", [NL, F], bf16)
        h1a_full = nc.dram_tensor("h1a_full", [NPAD, F], bf16,
                                  addr_space="Shared")

    rg = [list(range(ncores))]

    from contextlib import ExitStack
    with tile.TileContext(nc) as tc, ExitStack() as es:
        cpool = es.enter_context(tc.tile_pool(name="const", bufs=1))
        gpool = es.enter_context(tc.tile_pool(name="gather", bufs=bufs_gather))
        opool = es.enter_context(tc.tile_pool(name="onehot", bufs=2))
        wpool = es.enter_context(tc.tile_pool(name="work", bufs=3))
        ypool = es.enter_context(tc.tile_pool(name="yrow", bufs=2))
        pg = es.enter_context(tc.tile_pool(name="pagg", bufs=2, space="PSUM"))
        pt = es.enter_context(tc.tile_pool(name="ptr", bufs=2, space="PSUM"))
        ph = es.enter_context(tc.tile_pool(name="ph", bufs=2, space="PSUM"))

        # constants
        iota_sb = cpool.tile([P, P], f32, tag="iota")
        nc.sync.dma_start(iota_sb[:], iota_in.ap())
        ident = cpool.tile([P, P], bf16, tag="ident")
        make_identity(nc, ident[:])
        w_sb = {}
        for n in wnames:
            w_sb[n] = cpool.tile([F, F], bf16, tag=n, name=f"w_{n}")
            nc.sync.dma_start(w_sb[n][:], w_in[n].ap())
        b_sb = {}
        for n in ["b1", "b2", "blin"]:
            b_sb[n] = cpool.tile([F, 1], f32, tag=n, name=f"b_{n}")
            nc.sync.dma_start(b_sb[n][:], b_in[n].ap())
        inv_sb = cpool.tile([P, TPC], f32, tag="invcnt")
        nc.sync.dma_start(inv_sb[:], inv_in.ap())
        # persistent feature-major buffers
        xT_sb = cpool.tile([P, NL], bf16, tag="xT")
        nc.sync.dma_start(xT_sb[:], xt_in.ap())
        h1T_sb = cpool.tile([P, NL], bf16, tag="h1T")
        # whole gather-index and dstloc tables resident in SBUF: one DMA
        # each instead of per-group loads feeding the gather queues
        idx_sb = cpool.tile([P, max(m.IDXCOLS, 1)], i16, tag="idxall")
        nc.sync.dma_start(idx_sb[:], idx_in.ap())
        dl_sb = cpool.tile([P, max(m.SLOT_TOT, 1)], f32, tag="dlall")
        nc.sync.dma_start(dl_sb[:], dloc_in.ap())

        def layer(src_full, layer_idx):
            """layer_idx 1: x -> h1 (h1T into SBUF + h1 rows for AllGather).
            layer_idx 2: h1 -> h2 -> fused final linear -> y rows."""
            wl = w_sb["wl1t" if layer_idx == 1 else "wl2t"]
            wr = w_sb["wr1t" if layer_idx == 1 else "wr2t"]
            bb = b_sb["b1" if layer_idx == 1 else "b2"]
            rT_all = xT_sb if layer_idx == 1 else h1T_sb
            for gi, gr in enumerate(groups):
                # gathers for this group
                gbufs = []
                for k in range(K):
                    n = m.n_gk[gi][k]
                    if n == 0:
                        gbufs.append(None)
                        continue
                    gb = gpool.tile([P, n, F], bf16, tag=f"gbuf{k}")
                    if "lineardma" in ablate:
                        nc.sync.dma_start(
                            gb[:].rearrange("p s f -> p (s f)"),
                            src_full.ap()[:128 * n, :].rearrange(
                                "(p b) f -> p (b f)", p=128))
                    elif "gather" in ablate:
                        nc.vector.memset(gb[:, :1, :1], 0.0)
                    else:
                        cb = m.idxcols_gk[gi, k]
                        nc.gpsimd.dma_gather(
                            gb[:], src_full.ap()[k * CH:(k + 1) * CH, :],
                            idx_sb[:, cb:cb + n * (P // 16)],
                            n * P, n * P, F, single_packet=False,
                            queue_num=k)
                    gbufs.append(gb)
                # group-batched output buffers
                ng = len(gr)
                if layer_idx == 1:
                    hr_g = ypool.tile([P, ng, F], bf16, tag="h1rows")
                else:
                    yg = ypool.tile([P, ng, F], f32, tag="yrows")

                for ti, t in enumerate(gr):
                    rT = rT_all[:, t * P:(t + 1) * P]
                    # self/JK terms first: no gather dependency, keeps PE busy
                    phh = ph.tile([P, F], f32, tag="ph")
                    nc.tensor.matmul(phh[:], wr[:], rT,
                                     start=True, stop=False)
                    if layer_idx == 2:
                        pout = ph.tile([P, F], f32, tag="ph2")
                        nc.tensor.matmul(pout[:], w_sb["wlat"][:], rT,
                                         start=True, stop=False)
                    nst = int(m.slots_t[t])
                    # one-hot for all slots of tile t
                    if nst > 0:
                        oh = opool.tile([P, nst * P], bf16, tag="onehot")
                        if "onehot" in ablate:
                            nc.vector.memset(oh[:, :1], 0.0)
                        else:
                            dseg = dl_sb[:, int(m.dcol[t]):
                                         int(m.dcol[t]) + nst]
                            nc.vector.tensor_tensor(
                                out=oh[:].rearrange("p (s d) -> p s d", d=P),
                                in0=dseg[:, :, None].to_broadcast(
                                    [P, nst, P]),
                                in1=iota_sb[:, None, :].to_broadcast(
                                    [P, nst, P]),
                                op=mybir.AluOpType.is_equal)
                        pagg = pg.tile([P, F], f32, tag="pagg")
                        if "mm" not in ablate:
                            mm, last = 0, nst - 1
                            for k in range(K):
                                gb = gbufs[k]
                                for s in range(int(S[t, k])):
                                    col = int(m.koff[t, k]) + s
                                    slot = int(m.goff[t, k]) + s
                                    nc.tensor.matmul(
                                        pagg[:], oh[:, col * P:(col + 1) * P],
                                        gb[:, slot, :],
                                        start=(mm == 0), stop=(mm == last))
                                    mm += 1
                        else:
                            nc.tensor.matmul(pagg[:], oh[:, :P],
                                             gbufs[0][:, 0, :] if gbufs[0]
                                             is not None else oh[:, :P],
                                             start=True, stop=True)
                        aggm = wpool.tile([P, F], bf16, tag="aggm")
                        nc.vector.tensor_scalar_mul(
                            aggm[:], pagg[:], inv_sb[:, t:t + 1])
                    else:
                        aggm = wpool.tile([P, F], bf16, tag="aggm")
                        nc.vector.memset(aggm[:], 0.0)
                    ptr1 = pt.tile([P, F], bf16, tag="ptr")
                    nc.tensor.transpose(ptr1[:], aggm[:], ident[:])
                    aggT = wpool.tile([P, F], bf16, tag="aggT")
                    nc.scalar.copy(aggT[:], ptr1[:])
                    nc.tensor.matmul(phh[:], wl[:], aggT[:],
                                     start=False, stop=True)

                    if layer_idx == 1:
                        # h1T into persistent SBUF; h1 rows (bf16) for gather
                        hT = h1T_sb[:, t * P:(t + 1) * P]
                        nc.scalar.activation(
                            hT, phh[:], mybir.ActivationFunctionType.Relu,
                            bias=bb[:, :1])
                        ptr3 = pt.tile([P, F], bf16, tag="ptr")
                        nc.tensor.transpose(ptr3[:], hT, ident[:])
                        nc.scalar.copy(hr_g[:, ti, :], ptr3[:])
                    else:
                        hT = wpool.tile([P, F], bf16, tag="hT")
                        nc.scalar.activation(
                            hT[:], phh[:], mybir.ActivationFunctionType.Relu,
                            bias=bb[:, :1])
                        # fused final: [h1 h2] @ Wlin.T + blin, relu
                        nc.tensor.matmul(pout[:], w_sb["wlbt"][:], hT[:],
                                         start=False, stop=True)
                        oT = wpool.tile([P, F], bf16, tag="oT")
                        nc.scalar.activation(
                            oT[:], pout[:], mybir.ActivationFunctionType.Relu,
                            bias=b_sb["blin"][:, :1])
                        ptr4 = pt.tile([P, F], bf16, tag="ptr")
                        nc.tensor.transpose(ptr4[:], oT[:], ident[:])
                        nc.scalar.copy(yg[:, ti, :], ptr4[:])

                # group-batched stores
                if layer_idx == 1:
                    nc.sync.dma_start(
                        h1a_sh.ap()[gr[0] * P:(gr[-1] + 1) * P, :].rearrange(
                            "(t p) f -> p t f", p=P),
                        hr_g[:])
                else:
                    nc.sync.dma_start(
                        y_out.ap()[gr[0] * P:(gr[-1] + 1) * P, :].rearrange(
                            "(t p) f -> p t f", p=P),
                        yg[:])

        for _rep in range(repeat):
            if not single_core and "coll" not in ablate:
                nc.sync.dma_start(xa_sh.ap(), xr_in.ap())
                nc.gpsimd.collective_compute(
                    "AllGather", mybir.AluOpType.bypass, replica_groups=rg,
                    ins=[xa_sh.ap()], outs=[xa_full.ap()])
            layer(xa_full, 1)
            if not single_core and "coll" not in ablate:
                nc.gpsimd.collective_compute(
                    "AllGather", mybir.AluOpType.bypass, replica_groups=rg,
                    ins=[h1a_sh.ap()], outs=[h1a_full.ap()])
            layer(h1a_full, 2)

    nc.compile()
    return nc


# ----------------------------------------------------------------------------
# Full pipeline
# ----------------------------------------------------------------------------

def make_in_maps(m: Meta, inputs: dict):
    """Per-core input dicts from full inputs + meta."""
    x = np.asarray(inputs["x"], dtype=np.float32)
    TPC, ncores = m.TPC, m.ncores
    NL = TPC * P
    xpad = np.zeros((m.NPAD, F), dtype=np.float32)
    xpad[:m.N] = x
    xpad_bf = xpad.astype(ml_dtypes.bfloat16)
    iota = np.broadcast_to(np.arange(P, dtype=np.float32), (P, P)).copy()

    def wt(a):
        return np.ascontiguousarray(
            np.asarray(a, np.float32).T.astype(ml_dtypes.bfloat16))

    base = {
        "wl1t": wt(inputs["Wl1"]),
        "wr1t": wt(inputs["Wr1"]),
        "wl2t": wt(inputs["Wl2"]),
        "wr2t": wt(inputs["Wr2"]),
        "wlat": wt(np.asarray(inputs["Wlin"], np.float32)[:, :F]),
        "wlbt": wt(np.asarray(inputs["Wlin"], np.float32)[:, F:]),
        "b1": np.asarray(inputs["b1"], np.float32).reshape(F, 1),
        "b2": np.asarray(inputs["b2"], np.float32).reshape(F, 1),
        "blin": np.asarray(inputs["blin"], np.float32).reshape(F, 1),
        "iota": iota,
    }
    maps = []
    for c in range(ncores):
        d = dict(base)
        xs = xpad_bf[c * NL:(c + 1) * NL]
        d["x_rows"] = np.ascontiguousarray(xs)
        d["x_t"] = np.ascontiguousarray(xs.T)
        d["idx16"] = m.idx16[c]
        d["dstloc"] = m.dstloc[c]
        d["invcnt"] = m.invcnt[c]
        maps.append(d)
    return maps


def assemble_output(m: Meta, results):
    ys = [results[c]["y_local"] for c in range(m.ncores)]
    return np.concatenate(ys, axis=0)[:m.N].astype(np.float32)


# ----------------------------------------------------------------------------
# kernel() entry point
# ----------------------------------------------------------------------------

_N = 100000
_NCORES = 8
_GT = 7
_AGG = ml_dtypes.bfloat16

_cache = {}


def _get_program(edge_key, edge_index):
    if edge_key not in _cache:
        m = preprocess(edge_index, _N, _NCORES, _GT)
        nc = build_program(m, agg_np=_AGG)
        _cache[edge_key] = (m, nc)
    return _cache[edge_key]


def kernel(**inputs):
    from concourse.bass_utils import run_bass_kernel_spmd
    edge_index = np.asarray(inputs["edge_index"])
    assert edge_index.shape == (2, 1600000), edge_index.shape
    assert np.asarray(inputs["x"]).shape == (_N, 128)
    key = hash(edge_index.tobytes())
    m, nc = _get_program(key, edge_index)
    in_maps = make_in_maps(m, inputs)
    res = run_bass_kernel_spmd(nc, in_maps, list(range(_NCORES)))
    return assemble_output(m, [res.results[c] for c in range(_NCORES)])
